# revision 1
# baseline (speedup 1.0000x reference)
"""Trainium2 Bass kernel for nn_LinearTemporalSelfAttention (B=4,T=8192,D=512,H=8).

Sharding: 8 cores = B(4) x T-halves(2). Each core owns a (b, t-half) slab
(4096 x 512) end-to-end. Cross-core data is only the KV-state einsum
(sum over full T) and the emb projection (emb_W sharded over TE within a
pair) — both folded into ONE pair-wise AllReduce of a 134 KB buffer.

Math notes (exact up to fp assoc):
 - softmax shift-invariance: exp(k)/sum(exp(k)) without max-subtraction
   (values are O(1) after LN with 0.02-scale weights).
 - k-mask (+ -1e6) is replaced by masking v (v*mask) and computing the
   softmax-T denominator S = sum_t exp(k)*mask via an extra column of
   ones*mask appended to v in the same PE matmul.
 - gamma/beta of LN1 are folded into Wq/Wk/Wv (+ biases) on the host.
 - attn normalization (1/S) is applied to the tiny (64,8,65) state, and
   the q-softmax denominator (1/sum) is applied to y after the q@attn
   matmul — so the big T-sized tensors never need normalizing passes.
"""
import numpy as np
import ml_dtypes

B, T, D, H, TE = 4, 8192, 512, 8, 2048
Dh = D // H          # 64
EPS = 1e-5
NCORES = 8
TH = T // 2          # 4096 rows per core
P = 128
NT = TH // P         # 32 row tiles
KC = D // P          # 4 contraction chunks
TEH = TE // 2        # 1024 te rows per core
TEC = TEH // P       # 8 te chunks
CCU = 64 * H * (Dh + 1)     # 33280 floats of U_aug
CCN = CCU + 2 * D           # + emb partial

_CACHE: dict = {}


def _build(flags):
    has_bq, has_bk, has_bv, has_outb, has_embb = flags
    from contextlib import ExitStack
    import concourse.bass as bass
    import concourse.bacc as bacc
    import concourse.tile as tile
    import concourse.mybir as mybir
    from concourse.masks import make_identity

    f32 = mybir.dt.float32
    bf16 = mybir.dt.bfloat16
    Alu = mybir.AluOpType
    Act = mybir.ActivationFunctionType

    nc = bacc.Bacc("TRN2", target_bir_lowering=False, debug=False,
                   enable_asserts=True, num_devices=NCORES)

    x_in = nc.declare_dram_parameter("x", [TH, D], f32, isOutput=False)
    mk_in = nc.declare_dram_parameter("mask", [TH], f32, isOutput=False)
    emb_in = nc.declare_dram_parameter("embv", [TEH], f32, isOutput=False)
    wq_in = nc.declare_dram_parameter("wq", [KC, P, D], bf16, isOutput=False)
    wk_in = nc.declare_dram_parameter("wk", [KC, P, D], bf16, isOutput=False)
    wv_in = nc.declare_dram_parameter("wv", [KC, P, D], bf16, isOutput=False)
    wo_in = nc.declare_dram_parameter("wo", [KC, P, D], bf16, isOutput=False)
    we_in = nc.declare_dram_parameter("we", [TEC, P, 2 * D], bf16, isOutput=False)
    vec_in = nc.declare_dram_parameter("vecs", [1, 8, D], f32, isOutput=False)
    y_out = nc.declare_dram_parameter("y", [TH, D], f32, isOutput=True)

    PAIRS = [[0, 1], [2, 3], [4, 5], [6, 7]]

    with tile.TileContext(nc) as tc, ExitStack() as ctx:
        const = ctx.enter_context(tc.tile_pool(name="const", bufs=1))
        wpool = ctx.enter_context(tc.tile_pool(name="wpool", bufs=1))
        xstash = ctx.enter_context(tc.tile_pool(name="xstash", bufs=NT))
        qstash = ctx.enter_context(tc.tile_pool(name="qstash", bufs=NT))
        dramp = ctx.enter_context(tc.tile_pool(name="dram", bufs=1, space="DRAM"))

        ident = const.tile([P, P], bf16)
        make_identity(nc, ident)
        eps_t = const.tile([P, 1], f32)
        nc.vector.memset(eps_t, EPS)
        ones8 = const.tile([P, H, 1], bf16)
        nc.vector.memset(ones8, 1.0)
        ones_row = const.tile([1, P], bf16)
        nc.vector.memset(ones_row, 1.0)

        wq_s = wpool.tile([P, KC, D], bf16)
        nc.sync.dma_start(out=wq_s, in_=wq_in[:].rearrange("c p d -> p c d"))
        wk_s = wpool.tile([P, KC, D], bf16)
        nc.sync.dma_start(out=wk_s, in_=wk_in[:].rearrange("c p d -> p c d"))
        wv_s = wpool.tile([P, KC, D], bf16)
        nc.sync.dma_start(out=wv_s, in_=wv_in[:].rearrange("c p d -> p c d"))
        wo_s = wpool.tile([P, KC, D], bf16)
        nc.sync.dma_start(out=wo_s, in_=wo_in[:].rearrange("c p d -> p c d"))
        we_s = wpool.tile([P, TEC, 2 * D], bf16)
        nc.sync.dma_start(out=we_s, in_=we_in[:].rearrange("c p d -> p c d"))
        mask_s = wpool.tile([P, NT], f32)
        nc.sync.dma_start(out=mask_s, in_=mk_in[:].rearrange("(n p) -> p n", p=P))
        vec_s = wpool.tile([1, 8, D], f32)
        nc.sync.dma_start(out=vec_s, in_=vec_in[:])

        cc_in_t = dramp.tile([CCN], f32)
        cc_out_t = dramp.tile([CCN], f32)

        x_tiles = []
        q_tiles = []

        with ExitStack() as ctxA:
            work = ctxA.enter_context(tc.tile_pool(name="work", bufs=3))
            psA = ctxA.enter_context(tc.tile_pool(name="psA", bufs=2, space="PSUM"))
            psT = ctxA.enter_context(tc.tile_pool(name="psT", bufs=2, space="PSUM"))
            psU = ctxA.enter_context(tc.tile_pool(name="psU", bufs=1, space="PSUM"))
            embp = ctxA.enter_context(tc.tile_pool(name="embp", bufs=1))

            # ---- bias broadcast tiles (only when biases nonzero) ----
            # broadcast row -> [P, D] via PE: ones[1,P].T @ row[1,D]
            def bcast_row(row_idx, name):
                pb = psT.tile([P, D], f32, tag="pT")
                rbf = const.tile([1, D], bf16, tag="rbf_" + name)
                nc.vector.tensor_copy(out=rbf, in_=vec_s[:, row_idx, :])
                nc.tensor.matmul(out=pb, lhsT=ones_row, rhs=rbf,
                                 start=True, stop=True)
                bc = const.tile([P, D], f32, tag="bc_" + name)
                nc.scalar.copy(out=bc, in_=pb)
                return bc

            bq_bc = bcast_row(0, "bq") if has_bq else None
            bk_bc = bcast_row(1, "bk") if has_bk else None
            bv_bc = bcast_row(2, "bv") if has_bv else None
            ob_bc = bcast_row(3, "ob") if has_outb else None

            # ---- emb projection partial (this core's TE shard) ----
            embt = embp.tile([P, TEC], f32)
            nc.sync.dma_start(out=embt, in_=emb_in[:].rearrange("(c p) -> p c", p=P))
            embsg = embp.tile([P, TEC], f32)
            nc.scalar.activation(out=embsg, in_=embt, func=Act.Sigmoid)
            embs = embp.tile([P, TEC], bf16)
            nc.vector.tensor_mul(out=embs, in0=embt, in1=embsg)
            pe0 = psA.tile([1, D], f32, tag="pq")
            pe1 = psA.tile([1, D], f32, tag="pk")
            for j in range(TEC):
                nc.tensor.matmul(out=pe0, lhsT=embs[:, j:j + 1],
                                 rhs=we_s[:, j, 0:D],
                                 start=(j == 0), stop=(j == TEC - 1))
            for j in range(TEC):
                nc.tensor.matmul(out=pe1, lhsT=embs[:, j:j + 1],
                                 rhs=we_s[:, j, D:2 * D],
                                 start=(j == 0), stop=(j == TEC - 1))
            emb_part = embp.tile([1, 2 * D], f32)
            nc.scalar.copy(out=emb_part[:, 0:D], in_=pe0)
            nc.scalar.copy(out=emb_part[:, D:2 * D], in_=pe1)

            u0 = psU.tile([64, 4, Dh + 1], f32, tag="u0")
            u1 = psU.tile([64, 4, Dh + 1], f32, tag="u1")

            # ---- phase A: LN, QKV projections, exp, U accumulation ----
            # ACT uses ONLY the ln/exp table (rstd = exp(-0.5*ln(var+eps)))
            # so no ACT_TABLE_LOAD ever fires after the first one.
            for i in range(NT):
                xt = xstash.tile([P, D], f32, tag="x")
                x_tiles.append(xt)
                nc.sync.dma_start(out=xt, in_=x_in[i * P:(i + 1) * P, :])
                st = work.tile([P, 6], f32, tag="st")
                nc.vector.bn_stats(out=st, in_=xt)
                mv = work.tile([P, 2], f32, tag="mv")
                nc.vector.bn_aggr(out=mv, in_=st)
                sd = work.tile([P, 1], f32, tag="sd")
                nc.scalar.activation(out=sd, in_=mv[:, 1:2], func=Act.Ln,
                                     bias=eps_t)
                rstd = work.tile([P, 1], f32, tag="rstd")
                nc.scalar.activation(out=rstd, in_=sd, func=Act.Exp,
                                     scale=-0.5)
                xn = work.tile([P, D], bf16, tag="xn")
                nc.vector.tensor_scalar(out=xn, in0=xt, scalar1=mv[:, 0:1],
                                        scalar2=rstd, op0=Alu.subtract,
                                        op1=Alu.mult)
                xT = work.tile([P, KC, P], bf16, tag="xT")
                for j in range(KC):
                    nc.sync.dma_start(out=xT[:, j, :],
                                      in_=xn[:, j * P:(j + 1) * P],
                                      transpose=True)

                pq = psA.tile([P, D], f32, tag="pq")
                pk = psA.tile([P, D], f32, tag="pk")
                pv = psA.tile([P, D], f32, tag="pv")
                for j in range(KC):
                    nc.tensor.matmul(out=pq, lhsT=xT[:, j, :], rhs=wq_s[:, j, :],
                                     start=(j == 0), stop=(j == KC - 1))
                    nc.tensor.matmul(out=pk, lhsT=xT[:, j, :], rhs=wk_s[:, j, :],
                                     start=(j == 0), stop=(j == KC - 1))
                    nc.tensor.matmul(out=pv, lhsT=xT[:, j, :], rhs=wv_s[:, j, :],
                                     start=(j == 0), stop=(j == KC - 1))
                if has_bq:
                    nc.vector.tensor_add(out=pq, in0=pq, in1=bq_bc)
                if has_bk:
                    nc.vector.tensor_add(out=pk, in0=pk, in1=bk_bc)
                if has_bv:
                    nc.vector.tensor_add(out=pv, in0=pv, in1=bv_bc)

                qt = qstash.tile([P, D], bf16, tag="qt")
                q_tiles.append(qt)
                nc.scalar.activation(out=qt, in_=pq, func=Act.Exp)

                et = work.tile([P, D], bf16, tag="et")
                nc.scalar.activation(out=et, in_=pk, func=Act.Exp)

                va = work.tile([P, H, Dh + 1], bf16, tag="va")
                nc.vector.tensor_scalar_mul(
                    out=va[:, :, 0:Dh],
                    in0=pv[:].rearrange("p (h d) -> p h d", h=H),
                    scalar1=mask_s[:, i:i + 1])
                nc.vector.tensor_scalar_mul(out=va[:, :, Dh:Dh + 1], in0=ones8,
                                            scalar1=mask_s[:, i:i + 1])
                for h in range(H):
                    u = u0 if h < 4 else u1
                    # one accumulation group per PSUM bank: start clears the
                    # whole zero-region once; has_written bits make the first
                    # write to each head slot an overwrite, later ones adds.
                    nc.tensor.matmul(out=u[:, h % 4, :],
                                     lhsT=et[:, h * Dh:(h + 1) * Dh],
                                     rhs=va[:, h, :],
                                     start=(i == 0 and h % 4 == 0),
                                     stop=(i == NT - 1 and h % 4 == 3))

            # ---- ship partials through the pair AllReduce ----
            u_sb = embp.tile([64, H, Dh + 1], f32)
            nc.scalar.copy(out=u_sb[:, 0:4, :], in_=u0)
            nc.scalar.copy(out=u_sb[:, 4:8, :], in_=u1)
            nc.sync.dma_start(
                out=cc_in_t[0:CCU].rearrange("(p h f) -> p h f", p=64, h=H),
                in_=u_sb)
            nc.sync.dma_start(
                out=cc_in_t[CCU:CCN].rearrange("(a f) -> a f", a=1),
                in_=emb_part)
            nc.gpsimd.collective_compute(
                "AllReduce", Alu.add, replica_groups=PAIRS,
                ins=[cc_in_t[:]], outs=[cc_out_t[:]])

        # ---- phase B prologue: attn state + stylization vectors ----
        with ExitStack() as ctxB:
            workB = ctxB.enter_context(tc.tile_pool(name="workB", bufs=3))
            psB = ctxB.enter_context(tc.tile_pool(name="psB", bufs=2, space="PSUM"))
            embB = ctxB.enter_context(tc.tile_pool(name="embB", bufs=1))

            # U state duplicated on both partition halves; attn2 is the
            # block-diagonal per-pair layout for the merged y matmuls:
            # attn2[:, p, :] = [[attn_{2p}, 0], [0, attn_{2p+1}]]
            u_f = embB.tile([P, H, Dh + 1], f32)
            nc.sync.dma_start(
                out=u_f[0:64], in_=cc_out_t[0:CCU].rearrange(
                    "(p h f) -> p h f", p=64, h=H))
            nc.sync.dma_start(
                out=u_f[64:P], in_=cc_out_t[0:CCU].rearrange(
                    "(p h f) -> p h f", p=64, h=H))
            emb_f = embB.tile([1, 2 * D], f32)
            nc.sync.dma_start(
                out=emb_f, in_=cc_out_t[CCU:CCN].rearrange("(a f) -> a f", a=1))

            rs = embB.tile([P, H, 1], f32)
            nc.vector.reciprocal(out=rs, in_=u_f[:, :, Dh:Dh + 1])
            attn2 = embB.tile([P, KC, P], bf16)
            nc.gpsimd.memset(attn2, 0.0)
            for h in range(H):
                base = 64 * (h % 2)
                nc.vector.tensor_scalar_mul(
                    out=attn2[base:base + 64, h // 2, base:base + 64],
                    in0=u_f[base:base + 64, h, 0:Dh],
                    scalar1=rs[base:base + 64, h, :])

            srow = embB.tile([1, D], f32)
            shrow = embB.tile([1, D], f32)
            if has_embb:
                nc.vector.tensor_add(out=srow, in0=emb_f[:, 0:D],
                                     in1=vec_s[:, 6, :])
                nc.vector.tensor_add(out=shrow, in0=emb_f[:, D:2 * D],
                                     in1=vec_s[:, 7, :])
            else:
                nc.vector.tensor_copy(out=srow, in_=emb_f[:, 0:D])
                nc.vector.tensor_copy(out=shrow, in_=emb_f[:, D:2 * D])
            t1 = embB.tile([1, D], f32)
            nc.vector.tensor_scalar_add(out=t1, in0=srow, scalar1=1.0)
            arow = embB.tile([1, D], bf16)
            nc.vector.tensor_mul(out=arow, in0=t1, in1=vec_s[:, 4, :])
            crow_f = embB.tile([1, D], f32)
            nc.vector.tensor_mul(out=crow_f, in0=t1, in1=vec_s[:, 5, :])
            nc.vector.tensor_add(out=crow_f, in0=crow_f, in1=shrow)
            crow = embB.tile([1, D], bf16)
            nc.vector.tensor_copy(out=crow, in_=crow_f)

            # broadcast a,c rows to [P, D] via PE ones-outer-product
            pa = psB.tile([P, D], f32, tag="py")
            nc.tensor.matmul(out=pa, lhsT=ones_row, rhs=arow,
                             start=True, stop=True)
            a_bc = embB.tile([P, D], f32)
            nc.scalar.copy(out=a_bc, in_=pa)
            pc = psB.tile([P, D], f32, tag="py")
            nc.tensor.matmul(out=pc, lhsT=ones_row, rhs=crow,
                             start=True, stop=True)
            c_bc = embB.tile([P, D], f32)
            nc.scalar.copy(out=c_bc, in_=pc)

            # ---- phase B: y = q@attn, LN2, stylize, silu, out proj ----
            for i in range(NT):
                qt = q_tiles[i]
                qTt = workB.tile([P, KC, P], bf16, tag="qTt")
                for j in range(KC):
                    nc.sync.dma_start(out=qTt[:, j, :],
                                      in_=qt[:, j * P:(j + 1) * P],
                                      transpose=True)
                py = psB.tile([P, KC, P], f32, tag="py")
                for j in range(KC):
                    nc.tensor.matmul(out=py[:, j, :], lhsT=qTt[:, j, :],
                                     rhs=attn2[:, j, :], start=True, stop=True)
                # q-softmax denominator + evacuate py with ACT copy*scale
                qs = workB.tile([P, H, 1], f32, tag="qs")
                nc.vector.reduce_sum(
                    out=qs, in_=qt[:].rearrange("p (h d) -> p h d", h=H),
                    axis=mybir.AxisListType.X)
                rq = workB.tile([P, H], f32, tag="rq")
                nc.vector.reciprocal(out=rq, in_=qs[:, :, 0])
                py_flat = py[:].rearrange("p a b -> p (a b)")
                ysb = workB.tile([P, D], f32, tag="ysb")
                for h in range(H):
                    nc.scalar.activation(out=ysb[:, h * Dh:(h + 1) * Dh],
                                         in_=py_flat[:, h * Dh:(h + 1) * Dh],
                                         func=Act.Copy,
                                         scale=rq[:, h:h + 1])
                st2 = workB.tile([P, 6], f32, tag="st2")
                nc.vector.bn_stats(out=st2, in_=ysb)
                mv2 = workB.tile([P, 2], f32, tag="mv2")
                nc.vector.bn_aggr(out=mv2, in_=st2)
                sd2 = workB.tile([P, 1], f32, tag="sd2")
                nc.scalar.activation(out=sd2, in_=mv2[:, 1:2], func=Act.Ln,
                                     bias=eps_t)
                rstd2 = workB.tile([P, 1], f32, tag="rstd2")
                nc.scalar.activation(out=rstd2, in_=sd2, func=Act.Exp,
                                     scale=-0.5)
                # in-place: ysb -> z2 -> h1 (saves SBUF)
                nc.vector.tensor_scalar(out=ysb, in0=ysb, scalar1=mv2[:, 0:1],
                                        scalar2=rstd2, op0=Alu.subtract,
                                        op1=Alu.mult)
                nc.gpsimd.tensor_mul(out=ysb, in0=ysb, in1=a_bc)
                nc.gpsimd.tensor_add(out=ysb, in0=ysb, in1=c_bc)
                # silu(x) = x / (1 + exp(-x)) — keeps ACT on the exp table
                eneg = workB.tile([P, D], f32, tag="eneg")
                nc.scalar.activation(out=eneg, in_=ysb, func=Act.Exp,
                                     scale=-1.0)
                nc.gpsimd.tensor_scalar_add(out=eneg, in0=eneg, scalar1=1.0)
                nc.vector.reciprocal(out=eneg, in_=eneg)
                hs = workB.tile([P, D], bf16, tag="hs")
                nc.gpsimd.tensor_mul(out=hs, in0=ysb, in1=eneg)
                hT = workB.tile([P, KC, P], bf16, tag="hT")
                for j in range(KC):
                    nc.sync.dma_start(out=hT[:, j, :],
                                      in_=hs[:, j * P:(j + 1) * P],
                                      transpose=True)
                po = psB.tile([P, D], f32, tag="po")
                for j in range(KC):
                    nc.tensor.matmul(out=po, lhsT=hT[:, j, :],
                                     rhs=wo_s[:, j, :],
                                     start=(j == 0), stop=(j == KC - 1))
                osb = workB.tile([P, D], f32, tag="osb")
                nc.vector.tensor_add(out=osb, in0=po, in1=x_tiles[i])
                if has_outb:
                    nc.vector.tensor_add(out=osb, in0=osb, in1=ob_bc)
                nc.sync.dma_start(out=y_out[i * P:(i + 1) * P, :], in_=osb)

    nc.compile()
    return nc


def _prep(inputs, flags):
    bf = ml_dtypes.bfloat16
    x = np.asarray(inputs["x"], np.float32)
    emb = np.asarray(inputs["emb"], np.float32)
    src_mask = np.asarray(inputs["src_mask"], np.float32)
    gamma = np.asarray(inputs["gamma"], np.float32)
    beta = np.asarray(inputs["beta"], np.float32)
    gamma2 = np.asarray(inputs["gamma2"], np.float32)
    beta2 = np.asarray(inputs["beta2"], np.float32)
    emb_b = np.asarray(inputs["emb_b"], np.float32)
    out_b = np.asarray(inputs["out_b"], np.float32)

    def foldW(Wname):
        W = np.asarray(inputs[Wname], np.float32)
        return np.ascontiguousarray(
            (gamma[:, None] * W).astype(bf).reshape(KC, P, D))

    wq, wk, wv = foldW("Wq"), foldW("Wk"), foldW("Wv")
    wo = np.ascontiguousarray(
        np.asarray(inputs["out_W"], np.float32).astype(bf).reshape(KC, P, D))
    bq_f = np.asarray(inputs["bq"], np.float32) + beta @ np.asarray(inputs["Wq"], np.float32)
    bk_f = np.asarray(inputs["bk"], np.float32) + beta @ np.asarray(inputs["Wk"], np.float32)
    bv_f = np.asarray(inputs["bv"], np.float32) + beta @ np.asarray(inputs["Wv"], np.float32)
    vecs = np.ascontiguousarray(np.stack(
        [bq_f, bk_f, bv_f, out_b, gamma2, beta2, emb_b[:D], emb_b[D:]]
    ).astype(np.float32).reshape(1, 8, D))
    emb_W = np.asarray(inputs["emb_W"], np.float32)
    we_halves = [
        np.ascontiguousarray(
            emb_W[t * TEH:(t + 1) * TEH].astype(bf).reshape(TEC, P, 2 * D))
        for t in range(2)]

    in_maps = []
    for c in range(NCORES):
        b, th = c // 2, c % 2
        sl = slice(th * TH, (th + 1) * TH)
        in_maps.append({
            "x": np.ascontiguousarray(x[b, sl]),
            "mask": np.ascontiguousarray(src_mask[b, sl, 0]),
            "embv": np.ascontiguousarray(emb[b, th * TEH:(th + 1) * TEH]),
            "wq": wq, "wk": wk, "wv": wv, "wo": wo,
            "we": we_halves[th],
            "vecs": vecs,
        })
    return in_maps


def _flags(inputs):
    gamma = np.asarray(inputs["gamma"], np.float32)
    beta = np.asarray(inputs["beta"], np.float32)

    def nz(v):
        return bool(np.any(np.asarray(v) != 0))

    bq_f = np.asarray(inputs["bq"], np.float32) + beta @ np.asarray(inputs["Wq"], np.float32)
    bk_f = np.asarray(inputs["bk"], np.float32) + beta @ np.asarray(inputs["Wk"], np.float32)
    bv_f = np.asarray(inputs["bv"], np.float32) + beta @ np.asarray(inputs["Wv"], np.float32)
    return (nz(bq_f), nz(bk_f), nz(bv_f), nz(inputs["out_b"]), nz(inputs["emb_b"]))


def get_nc_and_inmaps(**inputs):
    flags = _flags(inputs)
    if flags not in _CACHE:
        _CACHE[flags] = _build(flags)
    return _CACHE[flags], _prep(inputs, flags)


def kernel(**inputs):
    from concourse.bass_utils import run_bass_kernel_spmd
    nc, in_maps = get_nc_and_inmaps(**inputs)
    res = run_bass_kernel_spmd(nc, in_maps, list(range(NCORES)))
    out = np.empty((B, T, D), np.float32)
    for c in range(NCORES):
        b, th = c // 2, c % 2
        out[b, th * TH:(th + 1) * TH] = res.results[c]["y"]
    return out



# revision 18
# speedup vs baseline: 2.1283x; 2.1283x over previous
"""Trainium2 Bass kernel for nn_LinearTemporalSelfAttention (B=4,T=8192,D=512,H=8).

Sharding: 8 cores = B(4) x T-halves(2). Each core owns a (b, t-half) slab
(4096 x 512) end-to-end; one pair-wise AllReduce carries the KV state + emb
projection partials.

v2 structural changes vs v1 baseline (562us):
 - all 384 DMA transposes -> PE transposes (is_transpose matmul), evacuated
   with one ACT copy per tile. DMA_TRANSPOSE occupied the Sync engine 472us.
 - q is computed TRANSPOSED in phase A (W-stationary matmuls) so phase B
   needs no q transpose; its softmax denominator comes from an extra
   ones-block matmul sharing the same LDWEIGHTS.
 - activation-table thrash fixed: Ln/Exp are forced into the combined
   natural_log_exp_and_others table (125 table loads = 160us in v1).
 - per-head y normalization via one tensor_tensor_reduce with broadcast AP
   (was 8 ACT Copy ops/tile), LN2 stats via its accum + ACT Square accum.
 - silu via exp + reciprocal_approx_fast (5x faster than reciprocal).
 - stylization via affine_mul_reduce; gpsimd only does two SBUF bf16 TTs.
 - U accumulation packs all 8 heads into ONE psum bank using col-group
   tile_position, freeing banks for double-buffering.
"""
import numpy as np
import ml_dtypes

B, T, D, H, TE = 4, 8192, 512, 8, 2048
Dh = D // H          # 64
EPS = 1e-5
NCORES = 8
TH = T // 2          # 4096 rows per core
P = 128
NT = TH // P         # 32 row tiles
KC = D // P          # 4 contraction chunks
TEH = TE // 2        # 1024 te rows per core
TEC = TEH // P       # 8 te chunks
CCU = P * KC * (Dh + 1)     # 33280 floats of U
CCN = CCU + 2 * D           # + emb partial

_CACHE: dict = {}
import os as _os
DEBUG = bool(int(_os.environ.get("BASSDBG", "0")))


def _patch_act_tables():
    """Force Ln and Exp to resolve to the combined ln+exp act table so the
    per-tile Ln<->Exp alternation never reloads tables (1.28us per load)."""
    import functools
    import concourse.hw_specs as hw
    if getattr(hw, "_ln_exp_combined_patch", False):
        return
    import concourse.mybir as mybir
    orig = hw.get_activation_tables

    @functools.cache
    def patched(arch):
        t = {k: set(v) for k, v in orig(arch).items()}
        EXP = mybir.ActivationFunctionType.Exp
        LN = mybir.ActivationFunctionType.Ln
        combined = [k for k, v in t.items() if EXP in v and LN in v]
        if combined:
            keep = combined[0]
            keepset = t[keep]
            for k, v in t.items():
                if k != keep:
                    # the combined table must be the unique resolution for
                    # every function it contains, so the load-insertion
                    # pass never alternates tables
                    v.difference_update(keepset)
        return t

    hw.get_activation_tables = patched
    import concourse.bacc as bacc_mod
    bacc_mod.get_activation_tables = patched
    try:
        import concourse.bass_interp as bi
        bi.get_activation_tables = patched
    except Exception:
        pass
    hw._ln_exp_combined_patch = True


def _build(flags):
    has_bq, has_bk, has_bv, has_outb, has_embb = flags
    from contextlib import ExitStack
    import concourse.bass as bass
    import concourse.bacc as bacc
    import concourse.tile as tile
    import concourse.mybir as mybir
    from concourse.masks import make_identity

    _patch_act_tables()

    f32 = mybir.dt.float32
    bf16 = mybir.dt.bfloat16
    Alu = mybir.AluOpType
    Act = mybir.ActivationFunctionType

    nc = bacc.Bacc("TRN2", target_bir_lowering=False, debug=False,
                   enable_asserts=True, num_devices=NCORES)

    x_in = nc.declare_dram_parameter("x", [TH, D], f32, isOutput=False)
    mk_in = nc.declare_dram_parameter("mask", [TH], f32, isOutput=False)
    emb_in = nc.declare_dram_parameter("embv", [TEH], f32, isOutput=False)
    wq_in = nc.declare_dram_parameter("wq", [KC, P, D], bf16, isOutput=False)
    wk_in = nc.declare_dram_parameter("wk", [KC, P, D], bf16, isOutput=False)
    wv_in = nc.declare_dram_parameter("wv", [KC, P, D], bf16, isOutput=False)
    wo_in = nc.declare_dram_parameter("wo", [KC, P, D], bf16, isOutput=False)
    we_in = nc.declare_dram_parameter("we", [TEC, P, 2 * D], bf16, isOutput=False)
    vec_in = nc.declare_dram_parameter("vecs", [1, 8, D], f32, isOutput=False)
    bqt_in = nc.declare_dram_parameter("bqt", [P, KC], f32, isOutput=False)
    y_out = nc.declare_dram_parameter("y", [TH, D], f32, isOutput=True)
    dbg = {}
    if DEBUG:
        for nm, shp in [("d_xn", [P, D]), ("d_xnT", [P, KC, P]),
                        ("d_qtT", [P, KC, P]), ("d_et", [P, D]),
                        ("d_va", [P, H, Dh + 1]), ("d_usb", [P, KC, Dh + 1]),
                        ("d_attn2", [P, KC, P]), ("d_qs", [P, KC, 2]),
                        ("d_py", [P, KC, P]), ("d_ysb", [P, D]),
                        ("d_h2", [P, D]), ("d_hs", [P, D]),
                        ("d_rstd2", [P, 1]), ("d_s1", [P, 1])]:
            dbg[nm] = nc.declare_dram_parameter(nm, shp, f32, isOutput=True)

    _dbgpool = [None]

    def dump(nm, ap, pool_, shape):
        if not DEBUG:
            return
        n = 1
        for s in shape[1:]:
            n *= s
        tmp = _dbgpool[0].tile([P, 520], f32, tag="dbg")
        src = ap if len(shape) <= 2 else ap.rearrange("p ... -> p (...)")
        dst = dbg[nm][:] if len(shape) <= 2 else \
            dbg[nm][:].rearrange("p ... -> p (...)")
        nc.vector.tensor_copy(out=tmp[:, 0:n], in_=src)
        nc.sync.dma_start(out=dst, in_=tmp[:, 0:n])

    PAIRS = [[0, 1], [2, 3], [4, 5], [6, 7]]

    with tile.TileContext(nc) as tc, ExitStack() as ctx:
        const = ctx.enter_context(tc.tile_pool(name="const", bufs=1))
        wpool = ctx.enter_context(tc.tile_pool(name="wpool", bufs=1))
        xstash = ctx.enter_context(tc.tile_pool(name="xstash", bufs=NT))
        qstash = ctx.enter_context(tc.tile_pool(name="qstash", bufs=NT))
        dramp = ctx.enter_context(tc.tile_pool(name="dram", bufs=1, space="DRAM"))

        if DEBUG:
            _dbgpool[0] = ctx.enter_context(tc.tile_pool(name="dbgp", bufs=1))
        ident = const.tile([P, P], bf16)
        make_identity(nc, ident)
        eps_t = const.tile([P, 1], f32)
        nc.vector.memset(eps_t, EPS)
        ones8 = const.tile([P, H, 1], bf16)
        nc.vector.memset(ones8, 1.0)
        ones_row = const.tile([1, P], bf16)
        nc.vector.memset(ones_row, 1.0)
        # ones block for q-softmax denominators: rows 0:64 -> col 0,
        # rows 64:128 -> col 1 (head pair within a 128-d chunk)
        onesblk = const.tile([P, 2], bf16)
        nc.vector.memset(onesblk, 0.0)
        nc.vector.memset(onesblk[0:64, 0:1], 1.0)
        nc.vector.memset(onesblk[64:P, 1:2], 1.0)

        wq_s = wpool.tile([P, KC, D], bf16)
        nc.sync.dma_start(out=wq_s, in_=wq_in[:].rearrange("c p d -> p c d"))
        wk_s = wpool.tile([P, KC, D], bf16)
        nc.sync.dma_start(out=wk_s, in_=wk_in[:].rearrange("c p d -> p c d"))
        wv_s = wpool.tile([P, KC, D], bf16)
        nc.sync.dma_start(out=wv_s, in_=wv_in[:].rearrange("c p d -> p c d"))
        wo_s = wpool.tile([P, KC, D], bf16)
        nc.sync.dma_start(out=wo_s, in_=wo_in[:].rearrange("c p d -> p c d"))
        we_s = wpool.tile([P, TEC, 2 * D], bf16)
        nc.sync.dma_start(out=we_s, in_=we_in[:].rearrange("c p d -> p c d"))
        mask_s = wpool.tile([P, NT], f32)
        nc.sync.dma_start(out=mask_s, in_=mk_in[:].rearrange("(n p) -> p n", p=P))
        vec_s = wpool.tile([1, 8, D], f32)
        nc.sync.dma_start(out=vec_s, in_=vec_in[:])
        bqt_s = wpool.tile([P, KC], f32)
        nc.sync.dma_start(out=bqt_s, in_=bqt_in[:])

        cc_in_t = dramp.tile([CCN], f32)
        cc_out_t = dramp.tile([CCN], f32)

        x_tiles = []
        q_tiles = []

        # ---- emb projection partial + bias broadcast tiles (own psum scope)
        embp = ctx.enter_context(tc.tile_pool(name="embp", bufs=1))
        with ExitStack() as ctxE:
            psE = ctxE.enter_context(tc.tile_pool(name="psE", bufs=1, space="PSUM"))

            def bcast_row(row_idx, name):
                pb = psE.tile([P, D], f32, tag="pe_b")
                rbf = const.tile([1, D], bf16, tag="rbf_" + name)
                nc.vector.tensor_copy(out=rbf, in_=vec_s[:, row_idx, :])
                nc.tensor.matmul(out=pb, lhsT=ones_row, rhs=rbf,
                                 start=True, stop=True)
                bc = const.tile([P, D], f32, tag="bc_" + name)
                nc.scalar.copy(out=bc, in_=pb)
                return bc

            bk_bc = bcast_row(1, "bk") if has_bk else None
            bv_bc = bcast_row(2, "bv") if has_bv else None
            ob_bc = bcast_row(3, "ob") if has_outb else None

            embt = embp.tile([P, TEC], f32)
            nc.sync.dma_start(out=embt, in_=emb_in[:].rearrange("(c p) -> p c", p=P))
            # silu(e) = e / (1 + exp(-e)) -- stays on the exp table
            een = embp.tile([P, TEC], f32)
            nc.scalar.activation(out=een, in_=embt, func=Act.Exp, scale=-1.0)
            nc.vector.tensor_scalar_add(out=een, in0=een, scalar1=1.0)
            nc.vector.reciprocal_approx_fast(out=een, in_=een)
            embs = embp.tile([P, TEC], bf16)
            nc.vector.tensor_mul(out=embs, in0=embt, in1=een)
            pe0 = psE.tile([1, D], f32, tag="pe0")
            pe1 = psE.tile([1, D], f32, tag="pe1")
            for j in range(TEC):
                nc.tensor.matmul(out=pe0, lhsT=embs[:, j:j + 1],
                                 rhs=we_s[:, j, 0:D],
                                 start=(j == 0), stop=(j == TEC - 1))
            for j in range(TEC):
                nc.tensor.matmul(out=pe1, lhsT=embs[:, j:j + 1],
                                 rhs=we_s[:, j, D:2 * D],
                                 start=(j == 0), stop=(j == TEC - 1))
            emb_part = embp.tile([1, 2 * D], f32)
            nc.scalar.copy(out=emb_part[:, 0:D], in_=pe0)
            nc.scalar.copy(out=emb_part[:, D:2 * D], in_=pe1)

        # ---- phase A: LN, QKV projections, exp, U accumulation ----
        with ExitStack() as ctxA:
            work = ctxA.enter_context(tc.tile_pool(name="work", bufs=3))
            psT = ctxA.enter_context(tc.tile_pool(name="psT", bufs=1, space="PSUM"))
            psKV = ctxA.enter_context(tc.tile_pool(name="psKV", bufs=2, space="PSUM"))
            psQ = ctxA.enter_context(tc.tile_pool(name="psQ", bufs=2, space="PSUM"))
            psU = ctxA.enter_context(tc.tile_pool(name="psU", bufs=1, space="PSUM"))

            # all 8 heads in ONE bank: partition half = head parity,
            # free slot = head pair
            u = psU.tile([P, KC, Dh + 1], f32, tag="u")

            for i in range(NT):
                xt = xstash.tile([P, D], f32, tag="x")
                x_tiles.append(xt)
                nc.sync.dma_start(out=xt, in_=x_in[i * P:(i + 1) * P, :])
                st = work.tile([P, 6], f32, tag="st")
                nc.vector.bn_stats(out=st, in_=xt)
                mv = work.tile([P, 2], f32, tag="mv")
                nc.vector.bn_aggr(out=mv, in_=st)
                sd = work.tile([P, 1], f32, tag="sd")
                nc.scalar.activation(out=sd, in_=mv[:, 1:2], func=Act.Ln,
                                     bias=eps_t)
                rstd = work.tile([P, 1], f32, tag="rstd")
                nc.scalar.activation(out=rstd, in_=sd, func=Act.Exp,
                                     scale=-0.5)
                xn = work.tile([P, D], bf16, tag="xn")
                nc.vector.tensor_scalar(out=xn, in0=xt, scalar1=mv[:, 0:1],
                                        scalar2=rstd, op0=Alu.subtract,
                                        op1=Alu.mult)
                pT = psT.tile([P, KC, P], bf16, tag="xnT")
                for j in range(KC):
                    nc.tensor.transpose(pT[:, j, :], xn[:, j * P:(j + 1) * P],
                                        ident)
                xnT = work.tile([P, KC, P], bf16, tag="xnT_s")
                nc.scalar.copy(out=xnT, in_=pT)
                if i == 0:
                    dump("d_xn", xn, work, [P, D])
                    dump("d_xnT", xnT, work, [P, KC, P])

                pk = psKV.tile([P, D], f32, tag="pk")
                pv = psKV.tile([P, D], f32, tag="pv")
                for j in range(KC):
                    nc.tensor.matmul(out=pk, lhsT=xnT[:, j, :], rhs=wk_s[:, j, :],
                                     start=(j == 0), stop=(j == KC - 1))
                for j in range(KC):
                    nc.tensor.matmul(out=pv, lhsT=xnT[:, j, :], rhs=wv_s[:, j, :],
                                     start=(j == 0), stop=(j == KC - 1))
                # q computed TRANSPOSED: pq[:, mc, t] = q^T[mc-block, t]
                pq = psQ.tile([P, KC, P], f32, tag="qT")
                for mc in range(KC):
                    for dc in range(KC):
                        nc.tensor.matmul(
                            out=pq[:, mc, :],
                            lhsT=wq_s[:, dc, mc * P:(mc + 1) * P],
                            rhs=xnT[:, dc, :],
                            start=(mc == 0 and dc == 0),
                            stop=(mc == KC - 1 and dc == KC - 1))
                if has_bk:
                    nc.vector.tensor_add(out=pk, in0=pk, in1=bk_bc)
                if has_bv:
                    nc.vector.tensor_add(out=pv, in0=pv, in1=bv_bc)
                if has_bq:
                    for mc in range(KC):
                        nc.vector.tensor_scalar_add(
                            out=pq[:, mc, :], in0=pq[:, mc, :],
                            scalar1=bqt_s[:, mc:mc + 1])

                qtT = qstash.tile([P, KC, P], bf16, tag="qt")
                q_tiles.append(qtT)
                nc.scalar.activation(out=qtT, in_=pq, func=Act.Exp)

                et = work.tile([P, D], bf16, tag="et")
                nc.scalar.activation(out=et, in_=pk, func=Act.Exp)
                va = work.tile([P, H, Dh + 1], bf16, tag="va")
                nc.scalar.activation(
                    out=va[:, :, 0:Dh],
                    in_=pv[:].rearrange("p (h d) -> p h d", h=H),
                    func=Act.Copy, scale=mask_s[:, i:i + 1])
                nc.vector.tensor_scalar_mul(out=va[:, :, Dh:Dh + 1], in0=ones8,
                                            scalar1=mask_s[:, i:i + 1])
                if i == 0:
                    dump("d_qtT", qtT, work, [P, KC, P])
                    dump("d_et", et, work, [P, D])
                    dump("d_va", va, work, [P, H, Dh + 1])
                for hp in range(KC):
                    nc.tensor.matmul(
                        out=u[0:64, hp, :],
                        lhsT=et[:, (2 * hp) * Dh:(2 * hp + 1) * Dh],
                        rhs=va[:, 2 * hp, :],
                        start=(i == 0 and hp == 0), stop=False,
                        tile_position=(0, 0))
                    nc.tensor.matmul(
                        out=u[64:P, hp, :],
                        lhsT=et[:, (2 * hp + 1) * Dh:(2 * hp + 2) * Dh],
                        rhs=va[:, 2 * hp + 1, :],
                        start=False,
                        stop=(i == NT - 1 and hp == KC - 1),
                        tile_position=(0, 64))

            u_sb = embp.tile([P, KC, Dh + 1], f32)
            nc.scalar.copy(out=u_sb, in_=u)
            dump("d_usb", u_sb, work, [P, KC, Dh + 1])
            nc.sync.dma_start(
                out=cc_in_t[0:CCU].rearrange("(p a f) -> p a f", p=P, a=KC),
                in_=u_sb)
            nc.sync.dma_start(
                out=cc_in_t[CCU:CCN].rearrange("(a f) -> a f", a=1),
                in_=emb_part)
            nc.gpsimd.collective_compute(
                "AllReduce", Alu.add, replica_groups=PAIRS,
                ins=[cc_in_t[:]], outs=[cc_out_t[:]])

        # ---- phase B: y = softmax(q)@attn, LN2, stylize, silu, out proj ----
        with ExitStack() as ctxB:
            workB = ctxB.enter_context(tc.tile_pool(name="workB", bufs=3))
            psY = ctxB.enter_context(tc.tile_pool(name="psY", bufs=2, space="PSUM"))
            embB = ctxB.enter_context(tc.tile_pool(name="embB", bufs=1))

            u_f = embB.tile([P, KC, Dh + 1], f32)
            nc.sync.dma_start(
                out=u_f, in_=cc_out_t[0:CCU].rearrange(
                    "(p a f) -> p a f", p=P, a=KC))
            emb_f = embB.tile([1, 2 * D], f32)
            nc.sync.dma_start(
                out=emb_f, in_=cc_out_t[CCU:CCN].rearrange("(a f) -> a f", a=1))

            rs = embB.tile([P, KC], f32)
            nc.vector.reciprocal_approx_fast(out=rs, in_=u_f[:, :, Dh])
            attn2 = embB.tile([P, KC, P], bf16)
            nc.gpsimd.memset(attn2, 0.0)
            for hp in range(KC):
                for hf in range(2):
                    b0 = 64 * hf
                    nc.vector.tensor_scalar_mul(
                        out=attn2[b0:b0 + 64, hp, b0:b0 + 64],
                        in0=u_f[b0:b0 + 64, hp, 0:Dh],
                        scalar1=rs[b0:b0 + 64, hp:hp + 1])

            srow = embB.tile([1, D], f32)
            shrow = embB.tile([1, D], f32)
            if has_embb:
                nc.vector.tensor_add(out=srow, in0=emb_f[:, 0:D],
                                     in1=vec_s[:, 6, :])
                nc.vector.tensor_add(out=shrow, in0=emb_f[:, D:2 * D],
                                     in1=vec_s[:, 7, :])
            else:
                nc.vector.tensor_copy(out=srow, in_=emb_f[:, 0:D])
                nc.vector.tensor_copy(out=shrow, in_=emb_f[:, D:2 * D])
            t1 = embB.tile([1, D], f32)
            nc.vector.tensor_scalar_add(out=t1, in0=srow, scalar1=1.0)
            arow = embB.tile([1, D], bf16)
            nc.vector.tensor_mul(out=arow, in0=t1, in1=vec_s[:, 4, :])
            crow_f = embB.tile([1, D], f32)
            nc.vector.tensor_mul(out=crow_f, in0=t1, in1=vec_s[:, 5, :])
            nc.vector.tensor_add(out=crow_f, in0=crow_f, in1=shrow)
            crow = embB.tile([1, D], bf16)
            nc.vector.tensor_copy(out=crow, in_=crow_f)

            pa = psY.tile([P, D], f32, tag="po")
            nc.tensor.matmul(out=pa, lhsT=ones_row, rhs=arow,
                             start=True, stop=True)
            a_bc = embB.tile([P, D], bf16)
            nc.scalar.copy(out=a_bc, in_=pa)
            pc = psY.tile([P, D], f32, tag="po")
            nc.tensor.matmul(out=pc, lhsT=ones_row, rhs=crow,
                             start=True, stop=True)
            c_bc = embB.tile([P, D], bf16)
            nc.scalar.copy(out=c_bc, in_=pc)

            dump("d_attn2", attn2, workB, [P, KC, P])
            inv_d = 1.0 / float(D)
            for i in range(NT):
                qtT = q_tiles[i]
                py = psY.tile([P, KC, P], f32, tag="py")
                qs = psY.tile([P, KC, 2], f32, tag="qs")
                for j in range(KC):
                    nc.tensor.matmul(out=py[:, j, :], lhsT=qtT[:, j, :],
                                     rhs=attn2[:, j, :],
                                     start=(j == 0), stop=(j == KC - 1))
                    nc.tensor.matmul(out=qs[:, j, :], lhsT=qtT[:, j, :],
                                     rhs=onesblk,
                                     start=(j == 0), stop=(j == KC - 1))
                rqt = workB.tile([P, H], f32, tag="rqt")
                nc.vector.reciprocal_approx_fast(
                    out=rqt, in_=qs[:].rearrange("p a b -> p (a b)"))
                ysb = workB.tile([P, D], bf16, tag="ysb")
                s1 = workB.tile([P, 1], f32, tag="s1")
                nc.vector.scalar_tensor_tensor(
                    out=ysb[:].rearrange("p (a d) -> p a d", a=H),
                    in0=py[:].rearrange("p a (h d) -> p (a h) d", h=2),
                    scalar=1.0,
                    in1=rqt[:, :, None].broadcast_to([P, H, Dh]),
                    op0=Alu.mult, op1=Alu.mult, accum_out=s1)
                if i == 0:
                    dump("d_py", py, workB, [P, KC, P])
                    dump("d_qs", qs, workB, [P, KC, 2])
                    dump("d_ysb", ysb, workB, [P, D])
                    dump("d_s1", s1, workB, [P, 1])
                sq = workB.tile([P, D], bf16, tag="sq")
                s2 = workB.tile([P, 1], f32, tag="s2")
                nc.scalar.activation(out=sq, in_=ysb, func=Act.Square,
                                     accum_out=s2)
                mu2 = workB.tile([P, 1], f32, tag="mu2")
                nc.vector.tensor_scalar_mul(out=mu2, in0=s1, scalar1=inv_d)
                t1b = workB.tile([P, 1], f32, tag="t1b")
                nc.vector.tensor_scalar_mul(out=t1b, in0=s2, scalar1=inv_d)
                t3 = workB.tile([P, 1], f32, tag="t3")
                nc.vector.scalar_tensor_tensor(
                    out=t3, in0=mu2, scalar=mu2, in1=t1b,
                    op0=Alu.mult, op1=Alu.subtract)
                sd2 = workB.tile([P, 1], f32, tag="sd2")
                nc.scalar.activation(out=sd2, in_=t3, func=Act.Ln,
                                     scale=-1.0, bias=eps_t)
                rstd2 = workB.tile([P, 1], f32, tag="rstd2")
                nc.scalar.activation(out=rstd2, in_=sd2, func=Act.Exp,
                                     scale=-0.5)
                nb = workB.tile([P, 1], f32, tag="nb")
                nc.vector.tensor_scalar(out=nb, in0=mu2, scalar1=rstd2,
                                        scalar2=-1.0, op0=Alu.mult,
                                        op1=Alu.mult)
                h2a = workB.tile([P, D], bf16, tag="h2a")
                dum = workB.tile([P, 1], f32, tag="dum")
                nc.vector.affine_mul_reduce(out=h2a, accum_out=dum, in0=ysb,
                                            in1=a_bc, scale=rstd2, bias=nb)
                h2 = workB.tile([P, D], bf16, tag="h2")
                nc.gpsimd.tensor_add(out=h2, in0=h2a, in1=c_bc)
                if i == 0:
                    dump("d_rstd2", rstd2, workB, [P, 1])
                    dump("d_h2", h2, workB, [P, D])
                eneg = workB.tile([P, D], f32, tag="eneg")
                nc.scalar.activation(out=eneg, in_=h2, func=Act.Exp,
                                     scale=-1.0)
                nc.vector.tensor_scalar_add(out=eneg, in0=eneg, scalar1=1.0)
                rec = workB.tile([P, D], f32, tag="rec")
                nc.vector.reciprocal_approx_fast(out=rec, in_=eneg)
                hs = workB.tile([P, D], bf16, tag="hs")
                nc.gpsimd.tensor_mul(out=hs, in0=h2, in1=rec)
                if i == 0:
                    dump("d_hs", hs, workB, [P, D])
                pT2 = psY.tile([P, KC, P], bf16, tag="hT")
                for j in range(KC):
                    nc.tensor.transpose(pT2[:, j, :], hs[:, j * P:(j + 1) * P],
                                        ident)
                hT = workB.tile([P, KC, P], bf16, tag="hT_s")
                nc.scalar.copy(out=hT, in_=pT2)
                po = psY.tile([P, D], f32, tag="po")
                for j in range(KC):
                    nc.tensor.matmul(out=po, lhsT=hT[:, j, :],
                                     rhs=wo_s[:, j, :],
                                     start=(j == 0), stop=(j == KC - 1))
                osb = workB.tile([P, D], f32, tag="osb")
                nc.vector.tensor_add(out=osb, in0=po, in1=x_tiles[i])
                if has_outb:
                    nc.vector.tensor_add(out=osb, in0=osb, in1=ob_bc)
                nc.sync.dma_start(out=y_out[i * P:(i + 1) * P, :], in_=osb)

    nc.compile()
    return nc


def _prep(inputs, flags):
    bf = ml_dtypes.bfloat16
    x = np.asarray(inputs["x"], np.float32)
    emb = np.asarray(inputs["emb"], np.float32)
    src_mask = np.asarray(inputs["src_mask"], np.float32)
    gamma = np.asarray(inputs["gamma"], np.float32)
    beta = np.asarray(inputs["beta"], np.float32)
    gamma2 = np.asarray(inputs["gamma2"], np.float32)
    beta2 = np.asarray(inputs["beta2"], np.float32)
    emb_b = np.asarray(inputs["emb_b"], np.float32)
    out_b = np.asarray(inputs["out_b"], np.float32)

    def foldW(Wname):
        W = np.asarray(inputs[Wname], np.float32)
        return np.ascontiguousarray(
            (gamma[:, None] * W).astype(bf).reshape(KC, P, D))

    wq, wk, wv = foldW("Wq"), foldW("Wk"), foldW("Wv")
    wo = np.ascontiguousarray(
        np.asarray(inputs["out_W"], np.float32).astype(bf).reshape(KC, P, D))
    bq_f = np.asarray(inputs["bq"], np.float32) + beta @ np.asarray(inputs["Wq"], np.float32)
    bk_f = np.asarray(inputs["bk"], np.float32) + beta @ np.asarray(inputs["Wk"], np.float32)
    bv_f = np.asarray(inputs["bv"], np.float32) + beta @ np.asarray(inputs["Wv"], np.float32)
    vecs = np.ascontiguousarray(np.stack(
        [bq_f, bk_f, bv_f, out_b, gamma2, beta2, emb_b[:D], emb_b[D:]]
    ).astype(np.float32).reshape(1, 8, D))
    bqt = np.ascontiguousarray(bq_f.reshape(KC, P).T.astype(np.float32))
    emb_W = np.asarray(inputs["emb_W"], np.float32)
    we_halves = [
        np.ascontiguousarray(
            emb_W[t * TEH:(t + 1) * TEH].astype(bf).reshape(TEC, P, 2 * D))
        for t in range(2)]

    in_maps = []
    for c in range(NCORES):
        b, th = c // 2, c % 2
        sl = slice(th * TH, (th + 1) * TH)
        in_maps.append({
            "x": np.ascontiguousarray(x[b, sl]),
            "mask": np.ascontiguousarray(src_mask[b, sl, 0]),
            "embv": np.ascontiguousarray(emb[b, th * TEH:(th + 1) * TEH]),
            "wq": wq, "wk": wk, "wv": wv, "wo": wo,
            "we": we_halves[th],
            "vecs": vecs, "bqt": bqt,
        })
    return in_maps


def _flags(inputs):
    beta = np.asarray(inputs["beta"], np.float32)

    def nz(v):
        return bool(np.any(np.asarray(v) != 0))

    bq_f = np.asarray(inputs["bq"], np.float32) + beta @ np.asarray(inputs["Wq"], np.float32)
    bk_f = np.asarray(inputs["bk"], np.float32) + beta @ np.asarray(inputs["Wk"], np.float32)
    bv_f = np.asarray(inputs["bv"], np.float32) + beta @ np.asarray(inputs["Wv"], np.float32)
    return (nz(bq_f), nz(bk_f), nz(bv_f), nz(inputs["out_b"]), nz(inputs["emb_b"]))


def get_nc_and_inmaps(**inputs):
    flags = _flags(inputs)
    if flags not in _CACHE:
        _CACHE[flags] = _build(flags)
    return _CACHE[flags], _prep(inputs, flags)


def kernel(**inputs):
    from concourse.bass_utils import run_bass_kernel_spmd
    nc, in_maps = get_nc_and_inmaps(**inputs)
    res = run_bass_kernel_spmd(nc, in_maps, list(range(NCORES)))
    out = np.empty((B, T, D), np.float32)
    for c in range(NCORES):
        b, th = c // 2, c % 2
        out[b, th * TH:(th + 1) * TH] = res.results[c]["y"]
    return out
